# revision 4
# baseline (speedup 1.0000x reference)
"""Trainium2 Bass kernel for batched tanh-attention flat-softmax.

Computes, per batch b:
    Q = query[b] @ W_query            # [S, DK]
    K = query[b] @ W_key              # [S, DK]
    s = tanh(Q @ K.T) * 10            # [S, S]
    s[diag] = -inf                    # (additive -1e8 in the reference)
    out[b]  = softmax(s.flatten())    # [S*S]

Sharding: data-parallel over batch across 8 NeuronCores (6 batches per
core); W_query/W_key replicated. No cross-core communication.

Since tanh(x)*10 is bounded in [-10, 10], softmax needs no max
subtraction: out = exp(10*tanh(s)) / sum(exp(10*tanh(s))), and the
diagonal is forced to exp(-1e5) == 0 by clamping the tanh output to
-1e4 on the diagonal before the exp.
"""

import numpy as np

import concourse.bass as bass
import concourse.bass_isa as bass_isa
import concourse.mybir as mybir
import concourse.tile as tile
from concourse import bacc
from concourse.bass_utils import run_bass_kernel_spmd
from concourse.masks import make_identity

# Problem shape (hardcoded; kernel.py must be self-contained).
B = 48
S = 1024
D = 128
DK = 64
N_CORES = 8
BPC = B // N_CORES  # batches per core
P = 128             # SBUF partitions
NQ = S // P         # q-row chunks per batch
F32 = mybir.dt.float32

TANH_CLIP = 10.0
DIAG_NEG = -1.0e4   # exp(10 * -1e4) underflows to exactly 0 in fp32


def build_bass() -> bass.Bass:
    nc = bacc.Bacc(None, target_bir_lowering=False)

    q_d = nc.dram_tensor("query", [BPC, S, D], F32, kind="ExternalInput")
    wq_d = nc.dram_tensor("W_query", [D, DK], F32, kind="ExternalInput")
    wk_d = nc.dram_tensor("W_key", [D, DK], F32, kind="ExternalInput")
    out_d = nc.dram_tensor("out", [BPC, S, S], F32, kind="ExternalOutput")

    with tile.TileContext(nc) as tc:
        with (
            tc.tile_pool(name="singles", bufs=1) as singles,
            tc.tile_pool(name="qload", bufs=2) as qload,
            tc.tile_pool(name="qtp", bufs=2) as qtp,
            tc.tile_pool(name="projsb", bufs=2) as projsb,
            tc.tile_pool(name="tbuf", bufs=2) as tbuf,
            tc.tile_pool(name="small", bufs=2) as small,
            tc.tile_pool(name="ps_tp", bufs=2, space="PSUM") as ps_tp,
            tc.tile_pool(name="ps_proj", bufs=1, space="PSUM") as ps_proj,
            tc.tile_pool(name="ps_sc", bufs=2, space="PSUM") as ps_sc,
        ):
            # --- one-time setup ---
            ident = singles.tile([P, P], F32)
            make_identity(nc, ident)

            # Diagonal clamp mask: min(t, mask) leaves off-diagonal t
            # untouched (mask=+3e38) and forces the diagonal to -1e4.
            dmask = singles.tile([P, P], F32)
            nc.vector.memset(dmask, 3.0e38)
            nc.gpsimd.affine_select(
                out=dmask,
                in_=dmask,
                compare_op=mybir.AluOpType.not_equal,
                fill=DIAG_NEG,
                base=0,
                pattern=[[-1, P]],
                channel_multiplier=1,
            )

            wq_sb = singles.tile([D, DK], F32)
            nc.sync.dma_start(wq_sb, wq_d[:, :])
            wk_sb = singles.tile([D, DK], F32)
            nc.sync.dma_start(wk_sb, wk_d[:, :])

            for b in range(BPC):
                # --- load query[b] as [p, n, d], s = n*128 + p ---
                q_sb = qload.tile([P, NQ, D], F32)
                nc.sync.dma_start(
                    q_sb, q_d[b].rearrange("(n p) d -> p n d", p=P)
                )

                # --- transpose to queryT [d, (n p)] = [128, 1024] ---
                qT = qtp.tile([D, NQ, P], F32)
                for n in range(NQ):
                    tp_ps = ps_tp.tile([P, P], F32)
                    nc.tensor.transpose(tp_ps, q_sb[:, n], ident)
                    nc.vector.tensor_copy(qT[:, n], tp_ps)

                # --- projections: QT = W_q.T @ queryT, KT = W_k.T @ queryT
                # out psum [64, 1024]; fp32 matmul free dim <= 512.
                qproj_ps = ps_proj.tile([DK, S], F32, tag="proj")
                nc.tensor.matmul(qproj_ps[:, 0:512], wq_sb, qT[:, 0:4])
                nc.tensor.matmul(qproj_ps[:, 512:1024], wq_sb, qT[:, 4:8])
                qTp = projsb.tile([DK, S], F32, tag="qTp")
                nc.vector.tensor_copy(qTp, qproj_ps)

                kproj_ps = ps_proj.tile([DK, S], F32, tag="proj")
                nc.tensor.matmul(kproj_ps[:, 0:512], wk_sb, qT[:, 0:4])
                nc.tensor.matmul(kproj_ps[:, 512:1024], wk_sb, qT[:, 4:8])
                kTp = projsb.tile([DK, S], F32, tag="kTp")
                nc.vector.tensor_copy(kTp, kproj_ps)

                # --- scores + tanh per 128-row chunk ---
                t_sb = tbuf.tile([P, NQ, S], F32, tag="t")
                for qc in range(NQ):
                    sc_ps = ps_sc.tile([P, S], F32, tag="sc")
                    lhsT = qTp[:, qc * P:(qc + 1) * P]
                    nc.tensor.matmul(sc_ps[:, 0:512], lhsT, kTp[:, 0:512])
                    nc.tensor.matmul(sc_ps[:, 512:1024], lhsT, kTp[:, 512:1024])
                    nc.scalar.activation(
                        out=t_sb[:, qc],
                        in_=sc_ps,
                        func=mybir.ActivationFunctionType.Tanh,
                    )
                    # clamp this chunk's diagonal block to -1e4
                    blk = t_sb[:, qc, qc * P:(qc + 1) * P]
                    nc.vector.tensor_tensor(blk, blk, dmask, mybir.AluOpType.min)

                # --- exp(10*t) in place, with per-partition row sums ---
                rs = small.tile([P, 1], F32, tag="rs")
                nc.scalar.activation(
                    out=t_sb,
                    in_=t_sb,
                    func=mybir.ActivationFunctionType.Exp,
                    scale=TANH_CLIP,
                    accum_out=rs,
                )

                # --- Z = sum over partitions; rz = 1/Z broadcast [128,1] ---
                zall = small.tile([P, 1], F32, tag="zall")
                nc.gpsimd.partition_all_reduce(
                    zall, rs, channels=P, reduce_op=bass_isa.ReduceOp.add
                )
                rz = small.tile([P, 1], F32, tag="rz")
                nc.vector.reciprocal(rz, zall)

                # --- normalize in place and store ---
                nc.vector.tensor_scalar_mul(t_sb, t_sb, rz)
                nc.sync.dma_start(
                    out_d[b].rearrange("(n p) s -> p n s", p=P), t_sb
                )

    nc.compile()
    return nc


_CACHED_NC = None


def kernel(**inputs: np.ndarray) -> np.ndarray:
    global _CACHED_NC
    query = np.ascontiguousarray(np.asarray(inputs["query"], dtype=np.float32))
    wq = np.ascontiguousarray(np.asarray(inputs["W_query"], dtype=np.float32))
    wk = np.ascontiguousarray(np.asarray(inputs["W_key"], dtype=np.float32))
    assert query.shape == (B, S, D), query.shape

    if _CACHED_NC is None:
        _CACHED_NC = build_bass()
    nc = _CACHED_NC

    in_maps = [
        {
            "query": query[c * BPC:(c + 1) * BPC],
            "W_query": wq,
            "W_key": wk,
        }
        for c in range(N_CORES)
    ]
    res = run_bass_kernel_spmd(nc, in_maps, core_ids=list(range(N_CORES)))
    out = np.concatenate(
        [r["out"].reshape(BPC, S * S) for r in res.results], axis=0
    )
    return out


# revision 9
# speedup vs baseline: 1.2929x; 1.2929x over previous
"""Trainium2 Bass kernel for batched tanh-attention flat-softmax.

Computes, per batch b:
    Q = query[b] @ W_query            # [S, DK]
    K = query[b] @ W_key              # [S, DK]
    s = tanh(Q @ K.T) * 10            # [S, S]
    s[diag] = -inf                    # (additive -1e8 in the reference)
    out[b]  = softmax(s.flatten())    # [S*S]

Sharding: data-parallel over batch across 8 NeuronCores (6 batches per
core); W_query/W_key replicated. No cross-core communication.

Since tanh(x)*10 is bounded in [-10, 10], softmax needs no max
subtraction: out = exp(10*tanh(s)) / sum(exp(10*tanh(s))), and the
diagonal is forced to exp(-1e5) == 0 by clamping the tanh output to
-1e4 on the diagonal before the exp.
"""

import numpy as np

import concourse.bass as bass
import concourse.bass_isa as bass_isa
import concourse.mybir as mybir
import concourse.tile as tile
from concourse import bacc
from concourse.bass_utils import run_bass_kernel_spmd
from concourse.masks import make_identity

# Problem shape (hardcoded; kernel.py must be self-contained).
B = 48
S = 1024
D = 128
DK = 64
N_CORES = 8
BPC = B // N_CORES  # batches per core
P = 128             # SBUF partitions
NQ = S // P         # q-row chunks per batch
F32 = mybir.dt.float32
F32R = mybir.dt.float32r  # fp32 bits, 1 cycle/row matmul mode (vs 4 for fp32)

TANH_CLIP = 10.0
DIAG_NEG = -1.0e4   # exp(10 * -1e4) underflows to exactly 0 in fp32


def build_bass() -> bass.Bass:
    nc = bacc.Bacc(None, target_bir_lowering=False)

    q_d = nc.dram_tensor("query", [BPC, S, D], F32, kind="ExternalInput")
    wq_d = nc.dram_tensor("W_query", [D, DK], F32, kind="ExternalInput")
    wk_d = nc.dram_tensor("W_key", [D, DK], F32, kind="ExternalInput")
    out_d = nc.dram_tensor("out", [BPC, S, S], F32, kind="ExternalOutput")

    with tile.TileContext(nc) as tc:
        with (
            tc.tile_pool(name="singles", bufs=1) as singles,
            tc.tile_pool(name="qload", bufs=2) as qload,
            tc.tile_pool(name="qtp", bufs=2) as qtp,
            tc.tile_pool(name="projsb", bufs=2) as projsb,
            tc.tile_pool(name="tbuf", bufs=2) as tbuf,
            tc.tile_pool(name="small", bufs=2) as small,
            tc.tile_pool(name="ps_tp", bufs=2, space="PSUM") as ps_tp,
            tc.tile_pool(name="ps_proj", bufs=1, space="PSUM") as ps_proj,
            tc.tile_pool(name="ps_sc", bufs=2, space="PSUM") as ps_sc,
        ):
            # --- one-time setup ---
            ident = singles.tile([P, P], F32)
            make_identity(nc, ident)

            # Diagonal clamp mask: min(t, mask) leaves off-diagonal t
            # untouched (mask=+3e38) and forces the diagonal to -1e4.
            dmask = singles.tile([P, P], F32)
            nc.vector.memset(dmask, 3.0e38)
            nc.gpsimd.affine_select(
                out=dmask,
                in_=dmask,
                compare_op=mybir.AluOpType.not_equal,
                fill=DIAG_NEG,
                base=0,
                pattern=[[-1, P]],
                channel_multiplier=1,
            )

            wq_stage = singles.tile([D, DK], F32)
            nc.sync.dma_start(wq_stage, wq_d[:, :])
            wk_stage = singles.tile([D, DK], F32)
            nc.sync.dma_start(wk_stage, wk_d[:, :])
            # round the weights to f32r once so PE can run 1-cycle/row matmuls
            wq_sb = singles.tile([D, DK], F32R)
            nc.vector.tensor_copy(wq_sb, wq_stage)
            wk_sb = singles.tile([D, DK], F32R)
            nc.vector.tensor_copy(wk_sb, wk_stage)

            for b in range(BPC):
                # --- load query[b] as [p, n, d], s = n*128 + p ---
                q_sb = qload.tile([P, NQ, D], F32)
                nc.sync.dma_start(
                    q_sb, q_d[b].rearrange("(n p) d -> p n d", p=P)
                )

                # --- transpose to queryT [d, (n p)] = [128, 1024] ---
                qT = qtp.tile([D, NQ, P], F32R)
                for n in range(NQ):
                    tp_ps = ps_tp.tile([P, P], F32)
                    nc.tensor.transpose(tp_ps, q_sb[:, n], ident)
                    nc.vector.tensor_copy(qT[:, n], tp_ps)

                # --- projections: QT = W_q.T @ queryT, KT = W_k.T @ queryT
                # out psum [64, 1024]; fp32 matmul free dim <= 512.
                qproj_ps = ps_proj.tile([DK, S], F32, tag="proj")
                nc.tensor.matmul(qproj_ps[:, 0:512], wq_sb, qT[:, 0:4])
                nc.tensor.matmul(qproj_ps[:, 512:1024], wq_sb, qT[:, 4:8])
                qTp = projsb.tile([DK, S], F32R, tag="qTp")
                nc.vector.tensor_copy(qTp, qproj_ps)

                kproj_ps = ps_proj.tile([DK, S], F32, tag="proj")
                nc.tensor.matmul(kproj_ps[:, 0:512], wk_sb, qT[:, 0:4])
                nc.tensor.matmul(kproj_ps[:, 512:1024], wk_sb, qT[:, 4:8])
                kTp = projsb.tile([DK, S], F32R, tag="kTp")
                nc.vector.tensor_copy(kTp, kproj_ps)

                # --- scores + tanh per 128-row chunk ---
                t_sb = tbuf.tile([P, NQ, S], F32, tag="t")
                for qc in range(NQ):
                    sc_ps = ps_sc.tile([P, S], F32, tag="sc")
                    lhsT = qTp[:, qc * P:(qc + 1) * P]
                    nc.tensor.matmul(sc_ps[:, 0:512], lhsT, kTp[:, 0:512])
                    nc.tensor.matmul(sc_ps[:, 512:1024], lhsT, kTp[:, 512:1024])
                    nc.scalar.activation(
                        out=t_sb[:, qc],
                        in_=sc_ps,
                        func=mybir.ActivationFunctionType.Tanh,
                    )
                    # clamp this chunk's diagonal block to -1e4
                    blk = t_sb[:, qc, qc * P:(qc + 1) * P]
                    nc.vector.tensor_tensor(blk, blk, dmask, mybir.AluOpType.min)

                # --- exp(10*t) in place, with per-partition row sums ---
                rs = small.tile([P, 1], F32, tag="rs")
                nc.scalar.activation(
                    out=t_sb,
                    in_=t_sb,
                    func=mybir.ActivationFunctionType.Exp,
                    scale=TANH_CLIP,
                    accum_out=rs,
                )

                # --- Z = sum over partitions; rz = 1/Z broadcast [128,1] ---
                zall = small.tile([P, 1], F32, tag="zall")
                nc.gpsimd.partition_all_reduce(
                    zall, rs, channels=P, reduce_op=bass_isa.ReduceOp.add
                )
                rz = small.tile([P, 1], F32, tag="rz")
                nc.vector.reciprocal(rz, zall)

                # --- normalize in place and store ---
                nc.vector.tensor_scalar_mul(t_sb, t_sb, rz)
                nc.sync.dma_start(
                    out_d[b].rearrange("(n p) s -> p n s", p=P), t_sb
                )

    nc.compile()
    return nc


_CACHED_NC = None


def kernel(**inputs: np.ndarray) -> np.ndarray:
    global _CACHED_NC
    query = np.ascontiguousarray(np.asarray(inputs["query"], dtype=np.float32))
    wq = np.ascontiguousarray(np.asarray(inputs["W_query"], dtype=np.float32))
    wk = np.ascontiguousarray(np.asarray(inputs["W_key"], dtype=np.float32))
    assert query.shape == (B, S, D), query.shape

    if _CACHED_NC is None:
        _CACHED_NC = build_bass()
    nc = _CACHED_NC

    in_maps = [
        {
            "query": query[c * BPC:(c + 1) * BPC],
            "W_query": wq,
            "W_key": wk,
        }
        for c in range(N_CORES)
    ]
    res = run_bass_kernel_spmd(nc, in_maps, core_ids=list(range(N_CORES)))
    out = np.concatenate(
        [r["out"].reshape(BPC, S * S) for r in res.results], axis=0
    )
    return out
